# revision 70
# baseline (speedup 1.0000x reference)
"""Embedding lookup + RMSNorm + tied logits projection on 8 trn2 NeuronCores.

Vocab-tensor-parallel, t-tile-major pipeline. TimelineSim ~523us/core vs
~650us for the first working version; the per-core floor is the bf16 matmul
roofline (~501us) plus ~10us of transposes.

  - Pad vocab 50257 -> 50272 = 8 * 6284; core c owns rows [c*6284,(c+1)*6284).
    final_norm folds into the projection weights on the host:
      logits[t,v] = rs[t] * (h[t,:] @ (w*fn).T)[t,v], rs = 1/sqrt(mean(h^2)+eps)
  - The full weight shard lives resident in SBUF (bf16, ~77KB/partition),
    streamed in 256-col chunks at kernel start. Logits are stored bf16
    (halves output DMA; host upcasts); rel err ~3e-3 vs the 2e-2 budget.
  - Per 128-token tile: indirect-gather h with f32->bf16 cast (gpsimd SWDGE;
    tiles 0-3 come host-pre-gathered to skip the idx->SWDGE cold-start),
    Square+accum_out -> ssq and Sqrt on Act + reciprocal on DVE -> rs, 6 bf16
    PE transposes h -> hnT via PSUM (1 cyc/row) with DVE copies to SBUF, 78
    matmuls (12x512+140 vocab chunks x 6 k-chunks, f32 PSUM, kk-inner), then
    PSUM->SBUF staging copies that apply rs as a per-partition scalar (DVE:
    vc0-7, Act: vc8-12, alternating on the last tile to shorten the drain)
    and one output DMA per vocab chunk (SP).
  - Tiles 0-2 run vc-major against the weight stream so each arriving weight
    chunk feeds 3 tiles of matmul work (PE is never supply-starved during the
    ~27us weight load); tiles 3-31 run t-major with phase-1 for tile g+1
    emitted ahead of the matmuls of tile g. Dummy junk transposes warm the PE
    p-state ramp during the initial DMA latency; a dummy Sqrt preloads the
    activation table. idx cols 4+ load after the weight chunks so phase-B
    gathers queue behind the weight stream; gathers run 4 tiles ahead.
  - Host assembles: concat shards over vocab, upcast bf16->f32, slice to
    50257, reshape [2,2048,V].
"""
import sys

sys.path.insert(0, "/opt/trn_rl_repo")

import numpy as np
import ml_dtypes

import concourse.mybir as mybir
import concourse.tile as tile
from concourse import bacc
from concourse.bass import IndirectOffsetOnAxis
from concourse.bass_utils import run_bass_kernel_spmd
from concourse.masks import make_identity

f32 = mybir.dt.float32
bf16 = mybir.dt.bfloat16
i32 = mybir.dt.int32

B, S, V, D = 2, 2048, 50257, 768
T = B * S                 # 4096 tokens
NC = 8                    # cores
VS = 6284                 # vocab shard per core (50272 padded)
KK = D // 128             # 6 k-chunks
NTT = T // 128            # 32 token tiles
# vocab chunks per tile: 12 x 512 + 1 x 140
VCS = [512] * 12 + [VS - 12 * 512]
VCO = [sum(VCS[:i]) for i in range(len(VCS))]
NVC = len(VCS)
SPLIT = 8                 # staging copies: vc 0..7 on DVE, 8..12 on Act
# output DMA pieces per tile (by vc index): [0,8) / [8,11) / [11,13)
PIECES = [(0, 8), (8, 11), (11, NVC)]
# weight-load chunks: 256 cols for fine-grained DMA interleaving
WCS = [256] * (VS // 256) + ([VS % 256] if VS % 256 else [])
WCO = [sum(WCS[:i]) for i in range(len(WCS))]
NA = 3                    # tiles processed vc-major during the weight stream
EPS = 1e-5

_cache = {}


def _build():
    nc = bacc.Bacc("TRN2", target_bir_lowering=False, debug=False, num_devices=NC)
    emb = nc.dram_tensor("emb", [V, D], f32, kind="ExternalInput")
    idxT = nc.dram_tensor("idxT", [128, NTT], i32, kind="ExternalInput")
    h03 = nc.dram_tensor("h03", [512, D], bf16, kind="ExternalInput")
    wt = nc.dram_tensor("wt", [128, KK, VS], bf16, kind="ExternalInput")
    out = nc.dram_tensor("out", [T, VS], bf16, kind="ExternalOutput")

    with tile.TileContext(nc) as tc:
        with (
            tc.tile_pool(name="const", bufs=1) as constp,
            tc.tile_pool(name="wtp", bufs=1) as wtp,
            tc.tile_pool(name="hp", bufs=6) as hp,
            tc.tile_pool(name="sqp", bufs=2) as sqp,
            tc.tile_pool(name="rsp", bufs=5) as rsp,
            tc.tile_pool(name="hntp", bufs=5) as hntp,
            tc.tile_pool(name="stp", bufs=3) as stp,
            tc.tile_pool(name="tps", bufs=3, space="PSUM") as tps,
            tc.tile_pool(name="mmp", bufs=5, space="PSUM") as mmp,
        ):
            ident = constp.tile([128, 128], bf16)
            make_identity(nc, ident[:])
            idx = constp.tile([128, NTT], i32)
            epsc = constp.tile([128, 1], f32)
            nc.vector.memset(epsc[:], EPS)
            junk = constp.tile([128, 128], f32)
            nc.vector.memset(junk[:], 1.0)
            # preload the Sqrt-containing activation table during idle time so
            # no LoadActFuncSet lands mid-pipeline (Sqrt/Square/Copy share it)
            actwarm = constp.tile([128, 1], f32)
            nc.scalar.activation(out=actwarm[:], in_=epsc[:],
                                 func=mybir.ActivationFunctionType.Sqrt)

            wts = wtp.tile([128, KK, VS], bf16)

            def load_weights(hook=None):
                for c in range(len(WCS)):
                    sl = slice(WCO[c], WCO[c] + WCS[c])
                    nc.sync.dma_start(out=wts[:, :, sl], in_=wt[:, :, sl])
                    if hook:
                        hook(c)

            # -- per-tile phase-1: gather + square/rsqrt + transpose to hnT
            def p1_gather(g):
                h = hp.tile([128, D], bf16, tag="h", name=f"h_{g}")
                if g < 4:
                    # tiles 0-3 come host-pre-gathered: plain DMA, no idx dep,
                    # so the pipeline starts without the SWDGE chain latency
                    nc.sync.dma_start(out=h[:], in_=h03[g * 128:(g + 1) * 128, :])
                else:
                    nc.gpsimd.indirect_dma_start(
                        out=h[:], out_offset=None, in_=emb[:],
                        in_offset=IndirectOffsetOnAxis(ap=idx[:, g:g + 1], axis=0),
                    )
                return h

            def p1_norm(g, h):
                sqd = sqp.tile([128, D], bf16, tag="sqd", name=f"sqd_{g}")
                ssq = rsp.tile([128, 1], f32, tag="ssq", name=f"ssq_{g}")
                nc.scalar.activation(out=sqd[:], in_=h[:],
                                     func=mybir.ActivationFunctionType.Square,
                                     accum_out=ssq[:])
                rms = rsp.tile([128, 1], f32, tag="rms", name=f"rms_{g}")
                nc.scalar.activation(out=rms[:], in_=ssq[:],
                                     func=mybir.ActivationFunctionType.Sqrt,
                                     bias=epsc[:, :1], scale=1.0 / D)
                rs = rsp.tile([128, 1], f32, tag="rs", name=f"rs_{g}")
                nc.vector.reciprocal(out=rs[:], in_=rms[:])
                return rs

            def p1_transpose(g, h):
                ptA = tps.tile([128, 512], bf16, tag="tp", name=f"ptA_{g}")
                ptB = tps.tile([128, 512], bf16, tag="tp", name=f"ptB_{g}")
                for kk in range(4):
                    nc.tensor.transpose(out=ptA[:, kk * 128:(kk + 1) * 128],
                                        in_=h[:, kk * 128:(kk + 1) * 128],
                                        identity=ident[:])
                for kk in range(4, KK):
                    nc.tensor.transpose(out=ptB[:, (kk - 4) * 128:(kk - 3) * 128],
                                        in_=h[:, kk * 128:(kk + 1) * 128],
                                        identity=ident[:])
                return ptA, ptB

            def p1_hnt(g, ptA, ptB):
                hnt = hntp.tile([128, KK, 128], bf16, tag="hnt", name=f"hnt_{g}")
                nc.vector.tensor_copy(
                    out=hnt[:, 0:4, :],
                    in_=ptA[:].rearrange("p (k t) -> p k t", k=4))
                nc.vector.tensor_copy(
                    out=hnt[:, 4:6, :],
                    in_=ptB[:, 0:256].rearrange("p (k t) -> p k t", k=2))
                return hnt

            # -- one (tile, vocab-chunk) matmul group: 6 matmuls + scaled
            #    PSUM->SBUF staging copy + output DMA piece
            def mm_group(g, c, hnt, rs, stg, eng):
                sl = slice(VCO[c], VCO[c] + VCS[c])
                ps = mmp.tile([128, 512], f32, tag="mm", name=f"mm_{g}_{c}")
                for kk in range(KK):
                    nc.tensor.matmul(
                        out=ps[:, :VCS[c]],
                        lhsT=hnt[:, kk, :],
                        rhs=wts[:, kk, sl],
                        start=(kk == 0), stop=(kk == KK - 1),
                    )
                if eng == "dve":
                    nc.vector.tensor_scalar_mul(
                        out=stg[:, sl], in0=ps[:, :VCS[c]], scalar1=rs[:, :1])
                else:
                    nc.scalar.activation(
                        out=stg[:, sl], in_=ps[:, :VCS[c]],
                        func=mybir.ActivationFunctionType.Copy,
                        scale=rs[:, :1])
                nc.sync.dma_start(out=out[g * 128:(g + 1) * 128, sl],
                                  in_=stg[:, sl])

            def p2(g, hnt, rs):
                stg = stp.tile([128, VS], bf16, tag="stg", name=f"stg_{g}")
                for c in range(NVC):
                    if g == NTT - 1:
                        eng = "dve" if c % 2 == 0 else "act"
                    else:
                        eng = "dve" if c < SPLIT else "act"
                    mm_group(g, c, hnt, rs, stg, eng)

            # -- emission --------------------------------------------------
            hs, rss, pts, hnts, stgs = {}, {}, {}, {}, {}
            hs[0] = p1_gather(0)

            def _wt_hook(c):
                if c == 1:
                    hs[1] = p1_gather(1)
                    hs[2] = p1_gather(2)
                elif c == 2:
                    hs[NA] = p1_gather(NA)

            load_weights(hook=_wt_hook)
            nc.sync.dma_start(out=idx[:, 4:NTT], in_=idxT[:, 4:NTT])

            # PE p-state warmup: dummy transposes so the array is at full
            # clock and busy until the first gather lands.
            def warm(n, base):
                for w in range(n):
                    wm = mmp.tile([128, 512], f32, tag="mm",
                                  name=f"warm_{base + w}")
                    nc.tensor.transpose(out=wm[:, 0:128], in_=junk[:],
                                        identity=junk[:])
            warm(20, 0)

            def p1_all(g):
                rss[g] = p1_norm(g, hs[g])
                pts[g] = p1_transpose(g, hs[g])
                hnts[g] = p1_hnt(g, *pts[g])

            # phase A: tiles 0..NA-1 vc-major against the weight stream
            for g in range(NA):
                stgs[g] = stp.tile([128, VS], bf16, tag="stg", name=f"stg_{g}")
            p1_all(0)
            p1_all(1)
            mm_group(0, 0, hnts[0], rss[0], stgs[0], "dve")
            mm_group(1, 0, hnts[1], rss[1], stgs[1], "act")
            p1_all(2)
            mm_group(2, 0, hnts[2], rss[2], stgs[2], "dve")
            for c in range(1, NVC):
                for g in range(NA):
                    eng = "dve" if (c * NA + g) % 2 == 0 else "act"
                    mm_group(g, c, hnts[g], rss[g], stgs[g], eng)
                if c == NVC - 3:
                    # prefetch phase-1 for the first phase-B tiles
                    hs[NA + 1] = p1_gather(NA + 1)
                    p1_all(NA)

            # phase B: tiles NA..NTT-1, t-major pipeline (gathers 4 ahead to
            # ride out the phase-A output-DMA backlog in the queue)
            next_g = NA + 2
            for g in range(NA, NTT):
                while next_g < min(g + 5, NTT):
                    hs[next_g] = p1_gather(next_g)
                    next_g += 1
                if g + 1 < NTT:
                    p1_all(g + 1)
                p2(g, hnts[g], rss[g])

    nc.compile()
    return nc


def _in_maps(input_sequence, embedding, final_norm, output_embedding):
    idx_flat = np.asarray(input_sequence).astype(np.int32).reshape(-1)
    idx_np = idx_flat.reshape(NTT, 128)
    idxT_np = np.ascontiguousarray(idx_np.T)                 # [128, NTT]
    emb_np = np.ascontiguousarray(np.asarray(embedding, dtype=np.float32))
    h03_np = np.ascontiguousarray(
        emb_np[idx_flat[:512]]).astype(ml_dtypes.bfloat16)   # tiles 0-3
    fn = np.asarray(final_norm, dtype=np.float32)
    w = np.asarray(output_embedding, dtype=np.float32) * fn[None, :]
    w_pad = np.zeros((NC * VS, D), dtype=np.float32)
    w_pad[:V] = w
    maps = []
    for c in range(NC):
        wc = w_pad[c * VS:(c + 1) * VS]                      # [VS, D]
        wtc = np.ascontiguousarray(
            wc.T.reshape(KK, 128, VS).transpose(1, 0, 2)).astype(ml_dtypes.bfloat16)
        maps.append({"emb": emb_np, "idxT": idxT_np, "h03": h03_np, "wt": wtc})
    return maps


def _run(in_maps, trace=False):
    if "nc" not in _cache:
        _cache["nc"] = _build()
    return run_bass_kernel_spmd(_cache["nc"], in_maps, list(range(NC)), trace=trace)


def kernel(input_sequence, embedding, final_norm, output_embedding):
    maps = _in_maps(input_sequence, embedding, final_norm, output_embedding)
    res = _run(maps)
    full = np.empty((T, NC * VS), dtype=np.float32)
    for c in range(NC):
        full[:, c * VS:(c + 1) * VS] = res.results[c]["out"]  # bf16 -> f32
    return np.ascontiguousarray(full[:, :V]).reshape(B, S, V)


# revision 71
# speedup vs baseline: 1.0311x; 1.0311x over previous
"""Embedding lookup + RMSNorm + tied logits projection on 8 trn2 NeuronCores.

Vocab-tensor-parallel, t-tile-major pipeline. TimelineSim ~523us/core vs
~650us for the first working version; the per-core floor is the bf16 matmul
roofline (~501us) plus ~10us of transposes.

  - Pad vocab 50257 -> 50272 = 8 * 6284; core c owns rows [c*6284,(c+1)*6284).
    final_norm folds into the projection weights on the host:
      logits[t,v] = rs[t] * (h[t,:] @ (w*fn).T)[t,v], rs = 1/sqrt(mean(h^2)+eps)
  - The full weight shard lives resident in SBUF (bf16, ~77KB/partition),
    streamed in 256-col chunks at kernel start. Logits are stored bf16
    (halves output DMA; host upcasts); rel err ~3e-3 vs the 2e-2 budget.
  - Per 128-token tile: indirect-gather h with f32->bf16 cast (gpsimd SWDGE;
    tiles 0-3 come host-pre-gathered to skip the idx->SWDGE cold-start),
    Square+accum_out -> ssq and Sqrt on Act + reciprocal on DVE -> rs, 6 bf16
    PE transposes h -> hnT via PSUM (1 cyc/row) with DVE copies to SBUF, 78
    matmuls (12x512+140 vocab chunks x 6 k-chunks, f32 PSUM, kk-inner), then
    PSUM->SBUF staging copies that apply rs as a per-partition scalar (DVE:
    vc0-7, Act: vc8-12, alternating on the last tile to shorten the drain)
    and one output DMA per vocab chunk (SP).
  - Tiles 0-2 run vc-major against the weight stream so each arriving weight
    chunk feeds 3 tiles of matmul work (PE is never supply-starved during the
    ~27us weight load); tiles 3-31 run t-major with phase-1 for tile g+1
    emitted ahead of the matmuls of tile g. Dummy junk transposes warm the PE
    p-state ramp during the initial DMA latency; a dummy Sqrt preloads the
    activation table. idx cols 4+ load after the weight chunks so phase-B
    gathers queue behind the weight stream; gathers run 4 tiles ahead.
  - Host assembles: concat shards over vocab, upcast bf16->f32, slice to
    50257, reshape [2,2048,V].
"""
import sys

sys.path.insert(0, "/opt/trn_rl_repo")

import numpy as np
import ml_dtypes

import concourse.mybir as mybir
import concourse.tile as tile
from concourse import bacc
from concourse.bass import IndirectOffsetOnAxis
from concourse.bass_utils import run_bass_kernel_spmd
from concourse.masks import make_identity

f32 = mybir.dt.float32
bf16 = mybir.dt.bfloat16
i32 = mybir.dt.int32

B, S, V, D = 2, 2048, 50257, 768
T = B * S                 # 4096 tokens
NC = 8                    # cores
VS = 6284                 # vocab shard per core (50272 padded)
KK = D // 128             # 6 k-chunks
NTT = T // 128            # 32 token tiles
# vocab chunks per tile: 12 x 512 + 1 x 140
VCS = [512] * 12 + [VS - 12 * 512]
VCO = [sum(VCS[:i]) for i in range(len(VCS))]
NVC = len(VCS)
SPLIT = 8                 # staging copies: vc 0..7 on DVE, 8..12 on Act
# output DMA pieces per tile (by vc index): [0,8) / [8,11) / [11,13)
PIECES = [(0, 8), (8, 11), (11, NVC)]
# weight-load chunks: 256 cols for fine-grained DMA interleaving
WCS = [256] * (VS // 256) + ([VS % 256] if VS % 256 else [])
WCO = [sum(WCS[:i]) for i in range(len(WCS))]
NA = 3                    # tiles processed vc-major during the weight stream
EPS = 1e-5

_cache = {}


def _build(ntt):
    nc = bacc.Bacc("TRN2", target_bir_lowering=False, debug=False, num_devices=NC)
    emb = nc.dram_tensor("emb", [V, D], f32, kind="ExternalInput")
    idxT = nc.dram_tensor("idxT", [128, ntt], i32, kind="ExternalInput")
    h03 = nc.dram_tensor("h03", [512, D], bf16, kind="ExternalInput")
    wt = nc.dram_tensor("wt", [128, KK, VS], bf16, kind="ExternalInput")
    out = nc.dram_tensor("out", [ntt * 128, VS], bf16, kind="ExternalOutput")

    with tile.TileContext(nc) as tc:
        with (
            tc.tile_pool(name="const", bufs=1) as constp,
            tc.tile_pool(name="wtp", bufs=1) as wtp,
            tc.tile_pool(name="hp", bufs=6) as hp,
            tc.tile_pool(name="sqp", bufs=2) as sqp,
            tc.tile_pool(name="rsp", bufs=5) as rsp,
            tc.tile_pool(name="hntp", bufs=5) as hntp,
            tc.tile_pool(name="stp", bufs=3) as stp,
            tc.tile_pool(name="tps", bufs=3, space="PSUM") as tps,
            tc.tile_pool(name="mmp", bufs=5, space="PSUM") as mmp,
        ):
            ident = constp.tile([128, 128], bf16)
            make_identity(nc, ident[:])
            idx = constp.tile([128, ntt], i32)
            epsc = constp.tile([128, 1], f32)
            nc.vector.memset(epsc[:], EPS)
            junk = constp.tile([128, 128], f32)
            nc.vector.memset(junk[:], 1.0)
            # preload the Sqrt-containing activation table during idle time so
            # no LoadActFuncSet lands mid-pipeline (Sqrt/Square/Copy share it)
            actwarm = constp.tile([128, 1], f32)
            nc.scalar.activation(out=actwarm[:], in_=epsc[:],
                                 func=mybir.ActivationFunctionType.Sqrt)

            wts = wtp.tile([128, KK, VS], bf16)

            def load_weights(hook=None):
                for c in range(len(WCS)):
                    sl = slice(WCO[c], WCO[c] + WCS[c])
                    nc.sync.dma_start(out=wts[:, :, sl], in_=wt[:, :, sl])
                    if hook:
                        hook(c)

            # -- per-tile phase-1: gather + square/rsqrt + transpose to hnT
            def p1_gather(g):
                h = hp.tile([128, D], bf16, tag="h", name=f"h_{g}")
                if g < 4:
                    # tiles 0-3 come host-pre-gathered: plain DMA, no idx dep,
                    # so the pipeline starts without the SWDGE chain latency
                    nc.sync.dma_start(out=h[:], in_=h03[g * 128:(g + 1) * 128, :])
                else:
                    nc.gpsimd.indirect_dma_start(
                        out=h[:], out_offset=None, in_=emb[:],
                        in_offset=IndirectOffsetOnAxis(ap=idx[:, g:g + 1], axis=0),
                    )
                return h

            def p1_norm(g, h):
                sqd = sqp.tile([128, D], bf16, tag="sqd", name=f"sqd_{g}")
                ssq = rsp.tile([128, 1], f32, tag="ssq", name=f"ssq_{g}")
                nc.scalar.activation(out=sqd[:], in_=h[:],
                                     func=mybir.ActivationFunctionType.Square,
                                     accum_out=ssq[:])
                rms = rsp.tile([128, 1], f32, tag="rms", name=f"rms_{g}")
                nc.scalar.activation(out=rms[:], in_=ssq[:],
                                     func=mybir.ActivationFunctionType.Sqrt,
                                     bias=epsc[:, :1], scale=1.0 / D)
                rs = rsp.tile([128, 1], f32, tag="rs", name=f"rs_{g}")
                nc.vector.reciprocal(out=rs[:], in_=rms[:])
                return rs

            def p1_transpose(g, h):
                ptA = tps.tile([128, 512], bf16, tag="tp", name=f"ptA_{g}")
                ptB = tps.tile([128, 512], bf16, tag="tp", name=f"ptB_{g}")
                for kk in range(4):
                    nc.tensor.transpose(out=ptA[:, kk * 128:(kk + 1) * 128],
                                        in_=h[:, kk * 128:(kk + 1) * 128],
                                        identity=ident[:])
                for kk in range(4, KK):
                    nc.tensor.transpose(out=ptB[:, (kk - 4) * 128:(kk - 3) * 128],
                                        in_=h[:, kk * 128:(kk + 1) * 128],
                                        identity=ident[:])
                return ptA, ptB

            def p1_hnt(g, ptA, ptB):
                hnt = hntp.tile([128, KK, 128], bf16, tag="hnt", name=f"hnt_{g}")
                nc.vector.tensor_copy(
                    out=hnt[:, 0:4, :],
                    in_=ptA[:].rearrange("p (k t) -> p k t", k=4))
                nc.vector.tensor_copy(
                    out=hnt[:, 4:6, :],
                    in_=ptB[:, 0:256].rearrange("p (k t) -> p k t", k=2))
                return hnt

            # -- one (tile, vocab-chunk) matmul group: 6 matmuls + scaled
            #    PSUM->SBUF staging copy + output DMA piece
            def mm_group(g, c, hnt, rs, stg, eng):
                sl = slice(VCO[c], VCO[c] + VCS[c])
                ps = mmp.tile([128, 512], f32, tag="mm", name=f"mm_{g}_{c}")
                for kk in range(KK):
                    nc.tensor.matmul(
                        out=ps[:, :VCS[c]],
                        lhsT=hnt[:, kk, :],
                        rhs=wts[:, kk, sl],
                        start=(kk == 0), stop=(kk == KK - 1),
                    )
                if eng == "dve":
                    nc.vector.tensor_scalar_mul(
                        out=stg[:, sl], in0=ps[:, :VCS[c]], scalar1=rs[:, :1])
                else:
                    nc.scalar.activation(
                        out=stg[:, sl], in_=ps[:, :VCS[c]],
                        func=mybir.ActivationFunctionType.Copy,
                        scale=rs[:, :1])
                nc.sync.dma_start(out=out[g * 128:(g + 1) * 128, sl],
                                  in_=stg[:, sl])

            def p2(g, hnt, rs):
                stg = stp.tile([128, VS], bf16, tag="stg", name=f"stg_{g}")
                for c in range(NVC):
                    if g == ntt - 1:
                        eng = "dve" if c % 2 == 0 else "act"
                    else:
                        eng = "dve" if c < SPLIT else "act"
                    mm_group(g, c, hnt, rs, stg, eng)

            # -- emission --------------------------------------------------
            hs, rss, pts, hnts, stgs = {}, {}, {}, {}, {}
            hs[0] = p1_gather(0)

            def _wt_hook(c):
                if c == 1:
                    hs[1] = p1_gather(1)
                    hs[2] = p1_gather(2)
                elif c == 2:
                    hs[NA] = p1_gather(NA)

            load_weights(hook=_wt_hook)
            nc.sync.dma_start(out=idx[:, 4:ntt], in_=idxT[:, 4:ntt])

            # PE p-state warmup: dummy transposes so the array is at full
            # clock and busy until the first gather lands.
            def warm(n, base):
                for w in range(n):
                    wm = mmp.tile([128, 512], f32, tag="mm",
                                  name=f"warm_{base + w}")
                    nc.tensor.transpose(out=wm[:, 0:128], in_=junk[:],
                                        identity=junk[:])
            warm(20, 0)

            def p1_all(g):
                rss[g] = p1_norm(g, hs[g])
                pts[g] = p1_transpose(g, hs[g])
                hnts[g] = p1_hnt(g, *pts[g])

            # phase A: tiles 0..NA-1 vc-major against the weight stream
            for g in range(NA):
                stgs[g] = stp.tile([128, VS], bf16, tag="stg", name=f"stg_{g}")
            p1_all(0)
            p1_all(1)
            mm_group(0, 0, hnts[0], rss[0], stgs[0], "dve")
            mm_group(1, 0, hnts[1], rss[1], stgs[1], "act")
            p1_all(2)
            mm_group(2, 0, hnts[2], rss[2], stgs[2], "dve")
            for c in range(1, NVC):
                for g in range(NA):
                    eng = "dve" if (c * NA + g) % 2 == 0 else "act"
                    mm_group(g, c, hnts[g], rss[g], stgs[g], eng)
                if c == NVC - 3:
                    # prefetch phase-1 for the first phase-B tiles
                    hs[NA + 1] = p1_gather(NA + 1)
                    p1_all(NA)

            # phase B: tiles NA..NTT-1, t-major pipeline (gathers 4 ahead to
            # ride out the phase-A output-DMA backlog in the queue)
            next_g = NA + 2
            for g in range(NA, ntt):
                while next_g < min(g + 5, ntt):
                    hs[next_g] = p1_gather(next_g)
                    next_g += 1
                if g + 1 < ntt:
                    p1_all(g + 1)
                p2(g, hnts[g], rss[g])

    nc.compile()
    return nc


def _in_maps(input_sequence, embedding, final_norm, output_embedding):
    # duplicate tokens have identical logits rows: compute only unique tokens
    # (padded to a 128 multiple), replicate rows on the host afterwards
    idx_flat = np.asarray(input_sequence).astype(np.int32).reshape(-1)
    uniq, inv = np.unique(idx_flat, return_inverse=True)
    ntt = max(5, -(-uniq.size // 128))                       # >=5 for phase A
    pad = ntt * 128 - uniq.size
    uniq_pad = np.concatenate([uniq, np.full(pad, uniq[0], np.int32)]).astype(np.int32)
    idx_np = uniq_pad.reshape(ntt, 128)
    idxT_np = np.ascontiguousarray(idx_np.T)                 # [128, ntt]
    emb_np = np.ascontiguousarray(np.asarray(embedding, dtype=np.float32))
    h03_np = np.ascontiguousarray(
        emb_np[uniq_pad[:512]]).astype(ml_dtypes.bfloat16)   # tiles 0-3
    fn = np.asarray(final_norm, dtype=np.float32)
    w = np.asarray(output_embedding, dtype=np.float32) * fn[None, :]
    w_pad = np.zeros((NC * VS, D), dtype=np.float32)
    w_pad[:V] = w
    maps = []
    for c in range(NC):
        wc = w_pad[c * VS:(c + 1) * VS]                      # [VS, D]
        wtc = np.ascontiguousarray(
            wc.T.reshape(KK, 128, VS).transpose(1, 0, 2)).astype(ml_dtypes.bfloat16)
        maps.append({"emb": emb_np, "idxT": idxT_np, "h03": h03_np, "wt": wtc})
    return maps, ntt, inv


def _run(in_maps, ntt, trace=False):
    if ntt not in _cache:
        _cache[ntt] = _build(ntt)
    return run_bass_kernel_spmd(_cache[ntt], in_maps, list(range(NC)), trace=trace)


def kernel(input_sequence, embedding, final_norm, output_embedding):
    maps, ntt, inv = _in_maps(input_sequence, embedding, final_norm,
                              output_embedding)
    res = _run(maps, ntt)
    comp = np.empty((ntt * 128, NC * VS), dtype=np.float32)
    for c in range(NC):
        comp[:, c * VS:(c + 1) * VS] = res.results[c]["out"]  # bf16 -> f32
    full = comp[inv]                                          # replicate dups
    return np.ascontiguousarray(full[:, :V]).reshape(B, S, V)
